# revision 32
# baseline (speedup 1.0000x reference)
"""Trainium2 Bass kernel for a dense transformer block (nn_Block_120259084502).

Contract: kernel(**inputs) takes the FULL unsharded inputs (numpy, fp32) and
returns the FULL output [4, 2048, 1024] fp32. Internally shards across 8
NeuronCores: core c handles batch c//2, query-token half c%2. Each core
receives its batch's full 2048 tokens (rolled so its own 1024 query tokens
come first) and computes K/V for all of them locally, so no collectives are
needed (attention context = full batch; softmax is order-invariant so the
roll is harmless).

Heavy matmuls run in fp8e4m3 with DoubleRow perf mode (2 contraction rows
per PE cell) and fp32 PSUM accumulation; attention scores stay bf16 so the
softmax input is accurate. Power-of-2 scales keep every fp8 operand in the
well-conditioned range and are folded into PSUM evictions / the softmax
ones-column. LayerNorm statistics and both residual adds stay fp32.
"""

import numpy as np
import ml_dtypes

import concourse.bacc as bacc
import concourse.tile as tile
from concourse import mybir
from concourse.bass_utils import run_bass_kernel_spmd
from concourse.masks import make_identity

bf16 = mybir.dt.bfloat16
f8e4 = mybir.dt.float8e4
f32 = mybir.dt.float32
AF = mybir.ActivationFunctionType
ALU = mybir.AluOpType
DR = mybir.MatmulPerfMode.DoubleRow

P = 128
B, T, E, H, D = 4, 2048, 1024, 16, 64
F = 4 * E                    # 4096 MLP hidden
TQ = T // 2                  # 1024 own query tokens per core
NE = E // P                  # 8 e-chunks
NC2 = NE // 2                # 4 DoubleRow e-chunk pairs
NPAIR = H // 2               # 8 head pairs
NST = T // P                 # 16 context-token tiles
NSP = NST // 2               # 8 context-tile pairs (DoubleRow attn.V)
NTS = TQ // P                # 8 own-token tiles
NF = F // P                  # 32 f-chunks
NF2 = NF // 2                # 16 DoubleRow f-chunk pairs
VW = D + 1                   # per-head V width incl. ones column
LN_EPS = 1e-5
NEG4LN2 = float(-4.0 * np.log(2.0))

# power-of-2 operand scales (host folds into weights; kernel folds inverse
# into PSUM evictions / softmax ones column)
SQ = 256.0   # wq scale (wq has 1/sqrt(D) folded -> tiny)
SK = 32.0    # wk scale
SV = 32.0    # wv scale
CS = 32.0    # catT carries 32*attn (fits fp8; proj evict divides out)
SP2 = 64.0   # proj_w scale
SU = 32.0    # up_w scale
SH = 8.0     # hid downscale: hid_fp8 = relu(.)/8
SD = 8.0     # down_w scale (SH*SD folds to 64; dn is true-scale: 8*8/64=1)

UP_F8C = 4        # of the 8 e-chunks feeding `up`, how many run fp8 DoubleRow
FP8_DOWN = False  # down-projection in fp8 DoubleRow (vs bf16)

_BUILD_CACHE = {}


class _Ctx:
    """Shared build state passed between phase emitters."""
    pass


def _emit_ln(g, xt, out_t):
    nc = g.nc
    st = g.stat.tile([P, 2, nc.vector.BN_STATS_DIM], f32, name="bnst")
    xv = xt.rearrange("p (s g) -> p s g", s=2)
    nc.vector.bn_stats(out=st[:, 0, :], in_=xv[:, 0, :])
    nc.vector.bn_stats(out=st[:, 1, :], in_=xv[:, 1, :])
    mv = g.stat.tile([P, nc.vector.BN_AGGR_DIM], f32, name="bnmv")
    nc.vector.bn_aggr(out=mv, in_=st)
    rstd = g.stat.tile([P, 1], f32, name="bnrs")
    nc.scalar.activation(out=rstd, in_=mv[:, 1:2], func=AF.Sqrt, bias=g.eps_t)
    nc.vector.reciprocal(out=rstd, in_=rstd)
    nc.vector.tensor_scalar(
        out=out_t, in0=xt, scalar1=mv[:, 0:1], scalar2=rstd,
        op0=ALU.subtract, op1=ALU.mult,
    )


def _emit_consts(g):
    nc, consts = g.nc, g.consts
    g.ident = consts.tile([P, P], bf16, name="ident")
    make_identity(nc, g.ident)
    g.eps_t = consts.tile([P, 1], f32, name="eps")
    nc.vector.memset(g.eps_t, LN_EPS)
    g.nexp_t = consts.tile([P, 1], f32, name="nexp")
    nc.vector.memset(g.nexp_t, NEG4LN2)
    g.rsc_t = consts.tile([P, 1], f32, name="rsc")
    nc.vector.memset(g.rsc_t, 1.0 / (SU * SH) if FP8_DOWN else 1.0 / SU)
    g.zero_t = consts.tile([P, 1], f32, name="zero")
    nc.vector.memset(g.zero_t, 0.0)
    g.ub_sb = consts.tile([P, NF], f32, name="ubsb")
    nc.sync.dma_start(out=g.ub_sb, in_=g.ub_d[:, :])
    if g.has_qb:
        g.qb_sb = consts.tile([P, NPAIR], f32, name="qbsb")
        nc.sync.dma_start(out=g.qb_sb, in_=g.qb_d[:, :])
        g.kb_sb = consts.tile([P, NPAIR], f32, name="kbsb")
        nc.sync.dma_start(out=g.kb_sb, in_=g.kb_d[:, :])
        g.vb_bc = consts.tile([P, 2 * E], bf16, name="vbbc")
        nc.gpsimd.dma_start(
            out=g.vb_bc, in_=g.vbrow_d.ap()[0:1, :].partition_broadcast(P)[:, 0, :]
        )
    if g.has_pb:
        g.pb_bc = consts.tile([P, E], f32, name="pbbc")
        nc.gpsimd.dma_start(
            out=g.pb_bc, in_=g.pbrow_d.ap()[0:1, :].partition_broadcast(P)[:, 0, :]
        )
    if g.has_db:
        g.db_bc = consts.tile([P, E], f32, name="dbbc")
        nc.gpsimd.dma_start(
            out=g.db_bc, in_=g.dbrow_d.ap()[0:1, :].partition_broadcast(P)[:, 0, :]
        )


def _emit_ln1_transpose(g, xkp, hp, tps):
    """Load x, LN1 -> h (fp8), PE-transpose into e-major hT2 via DMA."""
    nc = g.nc
    for i0 in range(0, NST, 4):
        hs = []
        for j in range(4):
            xt = xkp.tile([P, E], f32, name="xk")
            nc.sync.dma_start(out=xt, in_=g.xkv_d[(i0 + j) * P:(i0 + j + 1) * P, :])
            ht = hp.tile([P, E], bf16, name="h")
            _emit_ln(g, xt, ht)
            hs.append(ht)
        for c in range(NE):
            tp = tps.tile([P, 4 * P], bf16, name="tp")
            for j in range(4):
                nc.tensor.transpose(
                    tp[:, j * P:(j + 1) * P], hs[j][:, c * P:(c + 1) * P], g.ident
                )
            dst = g.hT2[c // 2][:, (c % 2) * T + i0 * P:(c % 2) * T + (i0 + 4) * P]
            nc.scalar.copy(out=dst, in_=tp)


def _emit_v(g, wvp, vps):
    """V in natural [s, d] layout for all heads, with ones column per head."""
    nc = g.nc
    wv_sb = []
    for c2 in range(NC2):
        w = wvp.tile([P, 2, E], f8e4, name=f"wv{c2}")
        nc.sync.dma_start(out=w, in_=g.wv_d[c2])
        wv_sb.append(w)
    for s in range(NST):
        sp, i = s // 2, s % 2
        if i == 0:
            nc.gpsimd.dma_start(
                out=g.va2[sp],
                in_=g.vrow_d.ap()[0:1, :].partition_broadcast(P)[:, 0, :],
            )
        pv = [vps.tile([P, 512], f32, name=f"pv{j}") for j in range(2)]
        for c2 in range(NC2):
            lhs = g.hT2[c2].rearrange("p (i t) -> p i t", i=2)[:, :, s * P:(s + 1) * P]
            for j in range(2):
                nc.tensor.matmul(
                    pv[j], lhs, wv_sb[c2][:, :, j * 512:(j + 1) * 512],
                    start=(c2 == 0), stop=(c2 == NC2 - 1), perf_mode=DR,
                )
        for j in range(2):
            dst = g.va2[sp].rearrange("p (i h c) -> p i h c", i=2, c=VW)[
                :, i, j * 8:(j + 1) * 8, 0:D
            ]
            src = pv[j].rearrange("p (h d) -> p h d", d=D)
            if g.has_qb:
                vb_view = g.vb_bc.rearrange("p (i h d) -> p i h d", i=2, d=D)[
                    :, i, j * 8:(j + 1) * 8, :
                ]
                nc.vector.scalar_tensor_tensor(
                    out=dst, in0=src, scalar=1.0 / SV, in1=vb_view,
                    op0=ALU.mult, op1=ALU.add,
                )
            else:
                nc.vector.tensor_scalar(
                    out=dst, in0=src, scalar1=1.0 / SV, scalar2=None, op0=ALU.mult
                )


def _emit_qkt_gen(g, p, wqkp, axps):
    """Q^T and K^T (fp8) for head pair p, as a generator so the emission is
    interleaved under the (ScalarE-bound) attention loop of pair p-1."""
    nc = g.nc
    qt, kt = g.qts[p], g.kts[p]
    wqs = []
    for c2 in range(NC2):
        w = wqkp.tile([P, 2, P], f8e4, name="wsl")
        nc.sync.dma_start(out=w, in_=g.wq_d[c2, p])
        wqs.append(w)
    for j in range(2):
        ps = axps.tile([P, 512], f32, name="axq", tag="ax")
        for c2 in range(NC2):
            rhs = g.hT2[c2].rearrange("p (i t) -> p i t", i=2)[
                :, :, j * 512:(j + 1) * 512
            ]
            nc.tensor.matmul(
                ps, wqs[c2], rhs,
                start=(c2 == 0), stop=(c2 == NC2 - 1), perf_mode=DR,
            )
        dst = qt[:, j * 512:(j + 1) * 512]
        if g.has_qb:
            nc.vector.tensor_scalar(
                out=dst, in0=ps, scalar1=1.0 / SQ, scalar2=g.qb_sb[:, p:p + 1],
                op0=ALU.mult, op1=ALU.add,
            )
        else:
            nc.vector.tensor_scalar(
                out=dst, in0=ps, scalar1=1.0 / SQ, scalar2=None, op0=ALU.mult
            )
        yield
    for sh in range(2):
        wks = []
        for c2 in range(NC2):
            w = wqkp.tile([P, 2, P], f8e4, name="wsl")
            nc.sync.dma_start(out=w, in_=g.wk_d[c2, p])
            wks.append(w)
        for j in range(2):
            s0 = (sh * 2 + j) * 512
            ps = axps.tile([P, 512], f32, name="axq", tag="ax")
            for c2 in range(NC2):
                rhs = g.hT2[c2].rearrange("p (i t) -> p i t", i=2)[:, :, s0:s0 + 512]
                nc.tensor.matmul(
                    ps, wks[c2], rhs,
                    start=(c2 == 0), stop=(c2 == NC2 - 1), perf_mode=DR,
                )
            dst = kt[:, s0:s0 + 512]
            if g.has_qb:
                nc.vector.tensor_scalar(
                    out=dst, in0=ps, scalar1=1.0 / SK,
                    scalar2=g.kb_sb[:, p:p + 1], op0=ALU.mult, op1=ALU.add,
                )
            else:
                nc.vector.tensor_scalar(
                    out=dst, in0=ps, scalar1=1.0 / SK, scalar2=None, op0=ALU.mult
                )
            yield


QB = 256                     # queries per pipeline block
NB = TQ // QB                # 4 blocks
NSPP = NST // 4              # 4 score/exp batches per (pair, block)


def _emit_attn_bp(g, p, tb, ptp, smp, scps, atps, filler, ndrive):
    """One (head pair, query block): scores -> exp(fp8) -> DoubleRow attn^T.

    `filler` emits next-pair QK (block 0) or previous-block MLP work between
    steps so the TensorE queue stays fed while this loop is ScalarE-bound."""
    nc = g.nc
    qt, kt = g.qts[p], g.kts[p]
    tcols = slice(tb * QB, (tb + 1) * QB)
    at = atps.tile([D + 1, 2 * QB], f32, name="at")
    for spp in range(NSPP):
        sc0 = scps.tile([P, 1024], f32, name="sc0")
        sc1 = scps.tile([P, 1024], f32, name="sc1")
        for i in range(4):
            s = 4 * spp + i
            scols = slice(s * P, (s + 1) * P)
            # S^T[s,t] = (K^T slice).T @ Q^T slice; the two heads live on
            # row-groups 0-63 / 64-127 so the matmuls pack concurrently.
            nc.tensor.matmul(sc0[:, i * QB:(i + 1) * QB],
                             kt[0:D, scols], qt[0:D, tcols],
                             start=True, stop=True)
            nc.tensor.matmul(sc1[:, i * QB:(i + 1) * QB],
                             kt[D:2 * D, scols], qt[D:2 * D, tcols],
                             start=True, stop=True)
        for _ in range(ndrive):
            next(filler, None)
        pt0 = ptp.tile([P, 1024], f8e4, name="pt0")
        pt1 = ptp.tile([P, 1024], f8e4, name="pt1")
        nc.scalar.activation(out=pt0, in_=sc0, func=AF.Exp, bias=g.nexp_t)
        nc.scalar.activation(out=pt1, in_=sc1, func=AF.Exp, bias=g.nexp_t)
        for k in range(2):
            sp = 2 * spp + k
            va_v = g.va2[sp].rearrange("p (i h c) -> p i h c", i=2, c=VW)
            nc.tensor.matmul(
                at[:, 0:QB], va_v[:, :, 2 * p, :],
                pt0[:, k * 512:(k + 1) * 512].rearrange("p (i t) -> p i t", i=2),
                start=(sp == 0), stop=(sp == NSP - 1), perf_mode=DR,
            )
            nc.tensor.matmul(
                at[:, QB:2 * QB], va_v[:, :, 2 * p + 1, :],
                pt1[:, k * 512:(k + 1) * 512].rearrange("p (i t) -> p i t", i=2),
                start=(sp == 0), stop=(sp == NSP - 1), perf_mode=DR,
            )
    se0 = smp.tile([1, QB], f32, name="se0")
    se1 = smp.tile([1, QB], f32, name="se1")
    nc.vector.reciprocal(out=se0, in_=at[D:D + 1, 0:QB])
    nc.vector.reciprocal(out=se1, in_=at[D:D + 1, QB:2 * QB])
    rb0 = smp.tile([D, QB], f32, name="rb0")
    rb1 = smp.tile([D, QB], f32, name="rb1")
    nc.gpsimd.partition_broadcast(rb0, se0)
    nc.gpsimd.partition_broadcast(rb1, se1)
    ct = g.catT2[p // 2]
    c0 = (p % 2) * TQ
    nc.vector.tensor_mul(out=ct[0:D, c0 + tb * QB:c0 + (tb + 1) * QB],
                         in0=at[0:D, 0:QB], in1=rb0)
    nc.vector.tensor_mul(out=ct[D:2 * D, c0 + tb * QB:c0 + (tb + 1) * QB],
                         in0=at[0:D, QB:2 * QB], in1=rb1)


def _emit_relu(g, hid_out, pu, f):
    """hid = relu(pu*scale + ub) -- DVE when ub==0, ScalarE otherwise."""
    nc = g.nc
    if g.has_ub:
        nc.scalar.activation(
            out=hid_out, in_=pu, func=AF.Relu,
            scale=g.rsc_t, bias=g.ub_sb[:, f:f + 1],
        )
    else:
        nc.vector.tensor_scalar(
            out=hid_out, in0=pu, scalar1=g.zero_t, scalar2=g.rsc_t,
            op0=ALU.max, op1=ALU.mult,
        )


def _emit_mlp_block_gen(g, b, xq2p, h2p, hidp, dwpp, outp, axps, dnps):
    """proj + LN2 + MLP for query block b, as a filler generator."""
    nc = g.nc
    h2s = []
    for ts in (2 * b, 2 * b + 1):
        trows = slice(ts * P, (ts + 1) * P)
        xres = xq2p.tile([P, E], f32, name="xres")
        nc.sync.dma_start(out=xres, in_=g.xkv_d[ts * P:(ts + 1) * P, :])
        x2 = g.x2_tiles[ts]
        for j in range(2):
            jc = slice(j * 512, (j + 1) * 512)
            psy = axps.tile([P, 512], f32, name="axy", tag="ax")
            for j2 in range(NC2):
                lhs = g.catT2[j2].rearrange("p (i t) -> p i t", i=2)[:, :, trows]
                nc.tensor.matmul(
                    psy, lhs, g.pw_sb[j2][:, :, jc],
                    start=(j2 == 0), stop=(j2 == NC2 - 1), perf_mode=DR,
                )
            if g.has_pb:
                nc.vector.scalar_tensor_tensor(
                    out=x2[:, jc], in0=psy, scalar=1.0 / (CS * SP2),
                    in1=g.pb_bc[:, jc], op0=ALU.mult, op1=ALU.add,
                )
                nc.vector.tensor_add(out=x2[:, jc], in0=x2[:, jc],
                                     in1=xres[:, jc])
            else:
                nc.vector.scalar_tensor_tensor(
                    out=x2[:, jc], in0=psy, scalar=1.0 / (CS * SP2),
                    in1=xres[:, jc], op0=ALU.mult, op1=ALU.add,
                )
            yield
        h2 = h2p.tile([P, E], bf16, name="h2")
        _emit_ln(g, x2, h2)
        h2s.append(h2)
        yield
    for c in range(NE):
        tp = axps.tile([P, 2 * P], bf16, name="axt", tag="ax")
        for j in range(2):
            nc.tensor.transpose(
                tp[:, j * P:(j + 1) * P], h2s[j][:, c * P:(c + 1) * P], g.ident
            )
        if c < UP_F8C:
            dst = g.h2T2[c // 2][
                :, (c % 2) * TQ + b * QB:(c % 2) * TQ + (b + 1) * QB]
        else:
            dst = g.h2Tb[c - UP_F8C][:, b * QB:(b + 1) * QB]
        nc.vector.tensor_copy(out=dst, in_=tp)
        if c % 2:
            yield
    qcols = slice(b * QB, (b + 1) * QB)
    hids = []
    nup = UP_F8C // 2 + (NE - UP_F8C)
    for f in range(NF):
        pu = axps.tile([P, QB], f32, name="axu", tag="ax")
        k = 0
        for c2 in range(UP_F8C // 2):
            rhs = g.h2T2[c2].rearrange("p (i t) -> p i t", i=2)[:, :, qcols]
            nc.tensor.matmul(
                pu, g.uw_sb[c2][:, :, f * P:(f + 1) * P], rhs,
                start=(k == 0), stop=(k == nup - 1), perf_mode=DR,
            )
            k += 1
        for cb in range(NE - UP_F8C):
            nc.tensor.matmul(
                pu, g.uwb_sb[cb][:, f * P:(f + 1) * P], g.h2Tb[cb][:, qcols],
                start=(k == 0), stop=(k == nup - 1),
            )
            k += 1
        hid = hidp.tile([P, QB], bf16, name=f"hid{f}")
        _emit_relu(g, hid, pu, f)
        hids.append(hid)
        if f % 2:
            yield
    for eh in range(2):
        ec = slice(eh * 512, (eh + 1) * 512)
        dnA = dnps.tile([P, 512], f32, name="dnA")
        dnB = dnps.tile([P, 512], f32, name="dnB")
        for f in range(NF):
            dwt = dwpp.tile([P, 512], bf16, name="dwt")
            nc.sync.dma_start(out=dwt, in_=g.dw_d[f, :, ec])
            nc.tensor.matmul(dnA, hids[f][:, 0:P], dwt,
                             start=(f == 0), stop=(f == NF - 1))
            nc.tensor.matmul(dnB, hids[f][:, P:2 * P], dwt,
                             start=(f == 0), stop=(f == NF - 1))
            if f % 4 == 3:
                yield
        for tt in range(2):
            ti = 2 * b + tt
            ot = outp.tile([P, 512], f32, name="ot")
            dn = dnA if tt == 0 else dnB
            nc.vector.tensor_add(out=ot, in0=dn, in1=g.x2_tiles[ti][:, ec])
            if g.has_db:
                nc.vector.tensor_add(out=ot, in0=ot, in1=g.db_bc[:, ec])
            nc.sync.dma_start(out=g.out_d[ti * P:(ti + 1) * P, ec], in_=ot)
        yield


def _emit_front(g, xkp, hp, tps, vps, wqkp, axps):
    """LN1 + transposes, interleaved with V and the first pair's QK."""
    nc = g.nc
    wv_sb = g.wv_sb
    qk0 = _emit_qkt_gen(g, 0, wqkp, axps)

    def emit_v(s):
        sp, i = s // 2, s % 2
        if i == 0:
            nc.gpsimd.dma_start(
                out=g.va2[sp],
                in_=g.vrow_d.ap()[0:1, :].partition_broadcast(P)[:, 0, :],
            )
        pv = [vps.tile([P, 512], f32, name=f"pv{j}") for j in range(2)]
        for c2 in range(NC2):
            lhs = g.hT2[c2].rearrange("p (i t) -> p i t", i=2)[
                :, :, s * P:(s + 1) * P]
            for j in range(2):
                nc.tensor.matmul(
                    pv[j], lhs, wv_sb[c2][:, :, j * 512:(j + 1) * 512],
                    start=(c2 == 0), stop=(c2 == NC2 - 1), perf_mode=DR,
                )
        for j in range(2):
            dst = g.va2[sp].rearrange("p (i h c) -> p i h c", i=2, c=VW)[
                :, i, j * 8:(j + 1) * 8, 0:D
            ]
            src = pv[j].rearrange("p (h d) -> p h d", d=D)
            if g.has_qb:
                vb_view = g.vb_bc.rearrange("p (i h d) -> p i h d", i=2, d=D)[
                    :, i, j * 8:(j + 1) * 8, :
                ]
                nc.vector.scalar_tensor_tensor(
                    out=dst, in0=src, scalar=1.0 / SV, in1=vb_view,
                    op0=ALU.mult, op1=ALU.add,
                )
            else:
                nc.vector.tensor_scalar(
                    out=dst, in0=src, scalar1=1.0 / SV, scalar2=None, op0=ALU.mult
                )

    for i0 in range(0, NST, 4):
        hs = []
        for j in range(4):
            xt = xkp.tile([P, E], f32, name="xk")
            nc.sync.dma_start(out=xt, in_=g.xkv_d[(i0 + j) * P:(i0 + j + 1) * P, :])
            ht = hp.tile([P, E], bf16, name="h")
            _emit_ln(g, xt, ht)
            hs.append(ht)
        for c in range(NE):
            tp = tps.tile([P, 4 * P], bf16, name="tp")
            for j in range(4):
                nc.tensor.transpose(
                    tp[:, j * P:(j + 1) * P], hs[j][:, c * P:(c + 1) * P], g.ident
                )
            dst = g.hT2[c // 2][:, (c % 2) * T + i0 * P:(c % 2) * T + (i0 + 4) * P]
            nc.scalar.copy(out=dst, in_=tp)
        for s in range(i0, i0 + 4):
            emit_v(s)
        if i0 == 4:
            # tokens 0-1023 transposed: Q (both halves) + K first half ready
            for _ in range(4):
                next(qk0, None)
        elif i0 == 12:
            for _ in qk0:
                pass


def _build(flags, reps=1):
    has_qb, has_pb, has_db, has_ub = flags
    nc = bacc.Bacc("TRN2", target_bir_lowering=False, debug=False, num_devices=8)

    g = _Ctx()
    g.nc = nc
    g.has_qb, g.has_pb, g.has_db, g.has_ub = flags
    g.xkv_d = nc.dram_tensor("xkv", [T, E], f32, kind="ExternalInput")
    g.wq_d = nc.dram_tensor("wq", [NC2, NPAIR, P, 2, P], f8e4, kind="ExternalInput")
    g.wk_d = nc.dram_tensor("wk", [NC2, NPAIR, P, 2, P], f8e4, kind="ExternalInput")
    g.wv_d = nc.dram_tensor("wv", [NC2, P, 2, E], f8e4, kind="ExternalInput")
    g.vrow_d = nc.dram_tensor("vrow", [1, 2 * H * VW], f8e4, kind="ExternalInput")
    g.pw_d = nc.dram_tensor("pw", [NC2, P, 2, E], f8e4, kind="ExternalInput")
    g.ub_d = nc.dram_tensor("ub", [P, NF], f32, kind="ExternalInput")
    if UP_F8C > 0:
        g.uw_d = nc.dram_tensor(
            "uw", [UP_F8C // 2, P, 2, F], f8e4, kind="ExternalInput")
    if UP_F8C < NE:
        g.uwb_d = nc.dram_tensor(
            "uwb", [NE - UP_F8C, P, F], bf16, kind="ExternalInput")
    if FP8_DOWN:
        g.dw_d = nc.dram_tensor("dw", [NF2, P, 2, E], f8e4, kind="ExternalInput")
    else:
        g.dw_d = nc.dram_tensor("dw", [NF, P, E], bf16, kind="ExternalInput")
    if has_qb:
        g.qb_d = nc.dram_tensor("qb", [P, NPAIR], f32, kind="ExternalInput")
        g.kb_d = nc.dram_tensor("kb", [P, NPAIR], f32, kind="ExternalInput")
        g.vbrow_d = nc.dram_tensor("vbrow", [1, 2 * E], bf16, kind="ExternalInput")
    if has_pb:
        g.pbrow_d = nc.dram_tensor("pbrow", [1, E], f32, kind="ExternalInput")
    if has_db:
        g.dbrow_d = nc.dram_tensor("dbrow", [1, E], f32, kind="ExternalInput")
    g.out_d = nc.dram_tensor("out", [TQ, E], f32, kind="ExternalOutput")

    with tile.TileContext(nc) as tc:
        with (
            tc.tile_pool(name="consts", bufs=1) as consts,
            tc.tile_pool(name="stat", bufs=4) as stat,
            tc.tile_pool(name="catp", bufs=1) as catp,
            tc.tile_pool(name="x2p", bufs=1) as x2p,
            tc.tile_pool(name="h2Tp", bufs=1) as h2Tp,
        ):
            g.consts, g.stat = consts, stat
            _emit_consts(g)
            for _rep in range(reps):
                _emit_all(g, tc, catp, x2p, h2Tp)

    nc.finalize()
    return nc


def _emit_all(g, tc, catp, x2p, h2Tp):
    nc = g.nc
    g.catT2 = [catp.tile([P, 2 * TQ], f8e4, name=f"catT{j}") for j in range(NC2)]
    g.x2_tiles = [x2p.tile([P, E], bf16, name=f"x2_{i}") for i in range(NTS)]
    g.h2T2 = [h2Tp.tile([P, 2 * TQ], f8e4, name=f"h2T{c}")
              for c in range(UP_F8C // 2)]
    g.h2Tb = [h2Tp.tile([P, TQ], bf16, name=f"h2Tb{c}")
              for c in range(NE - UP_F8C)]

    with (
        tc.tile_pool(name="vaug", bufs=1) as vap,
        tc.tile_pool(name="qtp", bufs=1) as qtp,
        tc.tile_pool(name="ktp", bufs=1) as ktp,
        tc.tile_pool(name="wup", bufs=1) as wup,
        tc.tile_pool(name="axps", bufs=1, space="PSUM") as axps,
    ):
        g.va2 = [vap.tile([P, 2 * H * VW], f8e4, name=f"va{s}")
                 for s in range(NSP)]
        g.qts = [qtp.tile([P, TQ], f8e4, name=f"qt{p}") for p in range(NPAIR)]
        g.kts = [ktp.tile([P, T], f8e4, name=f"kt{p}") for p in range(NPAIR)]
        # weight prefetches on the SWDGE queue (idle during the front)
        g.wv_sb = []
        for c2 in range(NC2):
            w = wup.tile([P, 2, E], f8e4, name=f"wv{c2}")
            nc.sync.dma_start(out=w, in_=g.wv_d[c2])
            g.wv_sb.append(w)
        g.pw_sb = []
        for c2 in range(NC2):
            w = wup.tile([P, 2, E], f8e4, name=f"pw{c2}")
            nc.gpsimd.dma_start(out=w, in_=g.pw_d[c2])
            g.pw_sb.append(w)
        g.uw_sb = []
        for c2 in range(UP_F8C // 2):
            w = wup.tile([P, 2, F], f8e4, name=f"uw{c2}")
            nc.gpsimd.dma_start(out=w, in_=g.uw_d[c2])
            g.uw_sb.append(w)
        g.uwb_sb = []
        for cb in range(NE - UP_F8C):
            w = wup.tile([P, F], bf16, name=f"uwb{cb}")
            nc.gpsimd.dma_start(out=w, in_=g.uwb_d[cb])
            g.uwb_sb.append(w)

        with (
            tc.tile_pool(name="ptp", bufs=2) as ptp,
            tc.tile_pool(name="smp", bufs=2) as smp,
            tc.tile_pool(name="xq2", bufs=2) as xq2p,
            tc.tile_pool(name="h2p", bufs=3) as h2p,
            tc.tile_pool(name="hidp", bufs=1) as hidp,
            tc.tile_pool(name="dwpp", bufs=4) as dwpp,
            tc.tile_pool(name="outp", bufs=2) as outp,
            tc.tile_pool(name="scps", bufs=1, space="PSUM") as scps,
            tc.tile_pool(name="atps", bufs=1, space="PSUM") as atps,
            tc.tile_pool(name="dnps", bufs=1, space="PSUM") as dnps,
        ):
            def _drain(gen):
                for _ in gen:
                    pass

            mlp_args = (xq2p, h2p, hidp, dwpp, outp, axps, dnps)
            with (
                tc.tile_pool(name="hTp", bufs=1) as hTp,
                tc.tile_pool(name="wqk", bufs=12) as wqkp,
            ):
                g.hT2 = [hTp.tile([P, 2 * T], f8e4, name=f"hT{c}")
                         for c in range(NC2)]
                with (
                    tc.tile_pool(name="xk", bufs=5) as xkp,
                    tc.tile_pool(name="hp", bufs=5) as hp,
                    tc.tile_pool(name="tps", bufs=2, space="PSUM") as tps,
                    tc.tile_pool(name="vps", bufs=2, space="PSUM") as vps,
                ):
                    with nc.named_scope("front"):
                        _emit_front(g, xkp, hp, tps, vps=vps,
                                    wqkp=wqkp, axps=axps)

                for p in range(NPAIR):
                    if p + 1 < NPAIR:
                        filler = _emit_qkt_gen(g, p + 1, wqkp, axps)
                    else:
                        filler = iter(())
                    with nc.named_scope(f"attn0_{p}"):
                        _emit_attn_bp(g, p, 0, ptp, smp, scps, atps,
                                      filler, 2)
                    _drain(filler)

            for tb in range(1, NB):
                filler = _emit_mlp_block_gen(g, tb - 1, *mlp_args)
                for p in range(NPAIR):
                    with nc.named_scope(f"attn{tb}_{p}"):
                        _emit_attn_bp(g, p, tb, ptp, smp, scps, atps,
                                      filler, 2)
                _drain(filler)
                with nc.named_scope("mlp_tail"):
                    _drain(_emit_mlp_block_gen(g, NB - 1, *mlp_args))


def _get_nc(flags, reps=1):
    key = (flags, reps)
    if key not in _BUILD_CACHE:
        _BUILD_CACHE[key] = _build(flags, reps)
    return _BUILD_CACHE[key]


def _to_f8(a):
    return np.ascontiguousarray(
        np.clip(a, -240.0, 240.0).astype(ml_dtypes.float8_e4m3)
    )


def _prep(x, Wq, Wk, Wv, proj_w, proj_b, ln1_g, ln1_b, ln2_g, ln2_b,
          up_w, up_b, down_w, down_b):
    """Host-side shard + weight fold/cast/layout. Returns (flags, in_maps)."""
    bfl = ml_dtypes.bfloat16
    x = np.ascontiguousarray(np.asarray(x, dtype=np.float32))
    Wq = np.asarray(Wq, np.float32)
    Wk = np.asarray(Wk, np.float32)
    Wv = np.asarray(Wv, np.float32)
    g1 = np.asarray(ln1_g, np.float32)
    b1 = np.asarray(ln1_b, np.float32)
    g2 = np.asarray(ln2_g, np.float32)
    b2 = np.asarray(ln2_b, np.float32)
    proj_w = np.asarray(proj_w, np.float32)
    up_w = np.asarray(up_w, np.float32)
    down_w = np.asarray(down_w, np.float32)

    # [H, E, D] -> [E, H*D]; fold attention scale into Q, LN1 gain into all
    wq_all = (Wq * (D ** -0.5)).transpose(1, 0, 2).reshape(E, E)
    wk_all = Wk.transpose(1, 0, 2).reshape(E, E)
    wv_all = Wv.transpose(1, 0, 2).reshape(E, E)
    qb_vec = b1 @ wq_all
    kb_vec = b1 @ wk_all
    vb_vec = b1 @ wv_all
    wq_f = g1[:, None] * wq_all * SQ
    wk_f = g1[:, None] * wk_all * SK
    wv_f = g1[:, None] * wv_all * SV

    def _pair_chunks(w):  # [E, E] -> [NC2, NPAIR, P, 2, P]
        return _to_f8(
            w.reshape(NC2, 2, P, NPAIR, P).transpose(0, 3, 2, 1, 4)
        )

    def _kx_chunks(w, ncols):  # [E_in, ncols] -> [NC2-ish, P, 2, ncols]
        n2 = w.shape[0] // (2 * P)
        return _to_f8(w.reshape(n2, 2, P, ncols).transpose(0, 2, 1, 3))

    vrow = np.zeros((1, 2 * H * VW), np.float32)
    vrow.reshape(2, H, VW)[:, :, D] = 1.0 / SV

    uw_f = g2[:, None] * up_w * SU
    ub_f = np.asarray(up_b, np.float32) + b2 @ up_w
    if FP8_DOWN:
        ub_f = ub_f / SH
        dw_h = _kx_chunks(down_w * SD, E)
    else:
        dw_h = np.ascontiguousarray(down_w.reshape(NF, P, E).astype(bfl))

    has_qb = bool(np.any(b1 != 0))
    has_pb = bool(np.any(np.asarray(proj_b) != 0))
    has_db = bool(np.any(np.asarray(down_b) != 0))
    has_ub = bool(np.any(ub_f != 0))
    flags = (has_qb, has_pb, has_db, has_ub)

    shared = {
        "wq": _pair_chunks(wq_f),
        "wk": _pair_chunks(wk_f),
        "wv": _kx_chunks(wv_f, E),
        "vrow": _to_f8(vrow),
        "pw": _kx_chunks(proj_w * SP2, E),
        "ub": np.ascontiguousarray(ub_f.reshape(NF, P).T.astype(np.float32)),
        "dw": dw_h,
    }
    if UP_F8C > 0:
        shared["uw"] = _kx_chunks(uw_f[:UP_F8C * P], F)
    if UP_F8C < NE:
        shared["uwb"] = np.ascontiguousarray(
            uw_f[UP_F8C * P:].reshape(NE - UP_F8C, P, F).astype(bfl))
    if has_qb:
        shared["qb"] = np.ascontiguousarray(
            qb_vec.reshape(NPAIR, P).T.astype(np.float32))
        shared["kb"] = np.ascontiguousarray(
            kb_vec.reshape(NPAIR, P).T.astype(np.float32))
        shared["vbrow"] = np.concatenate([vb_vec, vb_vec]).reshape(1, 2 * E).astype(bfl)
    if has_pb:
        shared["pbrow"] = np.asarray(proj_b, np.float32).reshape(1, E)
    if has_db:
        shared["dbrow"] = np.asarray(down_b, np.float32).reshape(1, E)

    in_maps = []
    for c in range(8):
        b, half = c // 2, c % 2
        xb = x[b]
        if half == 1:
            xb = np.concatenate([xb[TQ:], xb[:TQ]], axis=0)
        in_maps.append({"xkv": np.ascontiguousarray(xb), **shared})
    return flags, in_maps


def kernel(**inputs) -> np.ndarray:
    flags, in_maps = _prep(**inputs)
    nc = _get_nc(flags)
    res = run_bass_kernel_spmd(nc, in_maps, core_ids=list(range(8)))
    out = np.empty((B, T, E), np.float32)
    for c in range(8):
        b, half = c // 2, c % 2
        out[b, half * TQ:(half + 1) * TQ, :] = res.results[c]["out"]
    return out
